# revision 10
# baseline (speedup 1.0000x reference)
"""Trainium2 Bass kernel for nn_CustomAttention (MLA-style attention, f32).

Reference computation (B=1, S=2048, hidden=2048, H=16 heads):
    q = h @ Wq.T ; k = h @ Wk.T ; v = h @ Wv.T
    split q/k into nope (16x128) + rope (16x64), apply rotary to rope parts,
    scores = qn@kn.T/sqrt(128) + qr@kr.T/sqrt(64) ; attn = softmax(scores)
    out = (attn @ v) @ Wo.T

Sharding: tensor-parallel over heads.  Each of the 8 NeuronCores computes
2 heads end-to-end (its slice of Wq/Wk/Wv and Wo columns) and produces a
partial [S, hidden] output; host sums the 8 partials.

Device-side layout (per core, everything transposed so no on-chip
transposes of activations are needed):
    hT   [hid, S]   hidden_states transposed (moving operand of projections)
    wq   [hid, 384] = [nope_h0(128) nope_h1(128) rope_h0(64) rope_h1(64)]
                      columns of Wq_c.T, nope cols pre-scaled 1/sqrt(128),
                      rope cols pre-scaled 1/sqrt(64)
    wk   [hid, 384] same layout, unscaled
    wv   [hid, 256] Wv_c.T
    wo   [256, hid] Wo[:, c-slice].T
    ra/rb [128, S]  rotary coefficient tables (see _rope_tables)

Projections compute qT/kT ([dim, S]) and vT; v is PE-transposed back to
[S, dv].  Scores are computed transposed (sT[j, i] = k_j . q_i) so the
attn @ v contraction runs without transposing the softmax output; the
softmax row sums are computed with a ones-vector matmul, and the
normalization is folded into the PSUM eviction of attn@v.
All matmuls run as float32r (full-rate fp32 PE mode).
"""

import math
from contextlib import ExitStack

import numpy as np

import concourse.bass as bass
import concourse.tile as tile
from concourse import bacc, mybir
from concourse.bass_utils import run_bass_kernel_spmd
from concourse.kernels.tile_matmul import make_identity

H = 16
DN = 128
DR = 64
DV = 128
HID = 2048
S = 2048
ROPE_BASE = 10000.0

NCORES = 8
HPC = H // NCORES          # heads per core = 2
QC = HPC * (DN + DR)       # per-core q/k out dim = 384
VC = HPC * DV              # per-core v out dim = 256
P = 128
NK = HID // P              # 16 contraction chunks
FD = 512                   # matmul free dim / psum bank width (f32)
NCH = S // FD              # 4 column chunks of S
NT = S // P                # 16 row tiles of S

F32 = mybir.dt.float32
F32R = mybir.dt.float32r


def _build_program():
    nc = bacc.Bacc("TRN2", debug=False, num_devices=NCORES)

    hT = nc.dram_tensor("hT", [HID, S], F32R, kind="ExternalInput").ap()
    wq = nc.dram_tensor("wq", [HID, QC], F32R, kind="ExternalInput").ap()
    wk = nc.dram_tensor("wk", [HID, QC], F32R, kind="ExternalInput").ap()
    wv = nc.dram_tensor("wv", [HID, VC], F32R, kind="ExternalInput").ap()
    wo = nc.dram_tensor("wo", [VC, HID], F32R, kind="ExternalInput").ap()
    ra = nc.dram_tensor("ra", [P, S], F32, kind="ExternalInput").ap()
    rb = nc.dram_tensor("rb", [P, S], F32, kind="ExternalInput").ap()
    out = nc.dram_tensor("out", [S, HID], F32, kind="ExternalOutput").ap()

    with tile.TileContext(nc) as tc, ExitStack() as ctx:
        nc = tc.nc
        persist = ctx.enter_context(tc.tile_pool(name="persist", bufs=1))
        # qT/kT m-tiles: 0,1 = nope head0/head1 [128, S]; rope combined [128, S]
        q_nope = [persist.tile([P, S], F32R, tag=f"qn{h}", name=f"qn{h}") for h in range(HPC)]
        k_nope = [persist.tile([P, S], F32R, tag=f"kn{h}", name=f"kn{h}") for h in range(HPC)]
        q_rope = persist.tile([P, S], F32R, tag="qr")   # rows 0:64 h0, 64:128 h1
        k_rope = persist.tile([P, S], F32R, tag="kr")
        v_sb = persist.tile([P, NT, VC], F32R, tag="v")  # [j-in-tile, j-tile, dv]

        # ---------------- projection phase ----------------
        with ExitStack() as pctx:
            wpool = pctx.enter_context(tc.tile_pool(name="wpool", bufs=1))
            wq_sb = wpool.tile([P, NK, QC], F32R, tag="wq")
            wk_sb = wpool.tile([P, NK, QC], F32R, tag="wk")
            wv_sb = wpool.tile([P, NK, VC], F32R, tag="wv")
            nc.sync.dma_start(out=wq_sb[:], in_=wq.rearrange("(k p) m -> p k m", p=P))
            nc.sync.dma_start(out=wk_sb[:], in_=wk.rearrange("(k p) m -> p k m", p=P))
            nc.sync.dma_start(out=wv_sb[:], in_=wv.rearrange("(k p) m -> p k m", p=P))
            ra_sb = wpool.tile([P, S], F32, tag="ra")
            rb_sb = wpool.tile([P, S], F32, tag="rb")
            nc.sync.dma_start(out=ra_sb[:], in_=ra)
            nc.sync.dma_start(out=rb_sb[:], in_=rb)

            qr_raw = wpool.tile([P, S], F32, tag="qr_raw")
            kr_raw = wpool.tile([P, S], F32, tag="kr_raw")
            vt_sb = [wpool.tile([P, S], F32, tag=f"vt{h}", name=f"vt{h}") for h in range(HPC)]

            ht_pool = pctx.enter_context(tc.tile_pool(name="ht", bufs=3))

            # evict target for m-tile index: (tile, is_raw_rope)
            # m: 0,1 q nope; 2 q rope; 3,4 k nope; 5 k rope; 6,7 vT
            def evict_target(m, n):
                sl = bass.ts(n, FD)
                if m < 2:
                    return q_nope[m][:, sl]
                if m == 2:
                    return qr_raw[:, sl]
                if m < 5:
                    return k_nope[m - 3][:, sl]
                if m == 5:
                    return kr_raw[:, sl]
                return vt_sb[m - 6][:, sl]

            def lhsT_for(m, k):
                if m < 3:
                    return wq_sb[:, k, bass.ts(m, P)]
                if m < 6:
                    return wk_sb[:, k, bass.ts(m - 3, P)]
                return wv_sb[:, k, bass.ts(m - 6, P)]

            with tc.tile_pool(name="proj_psum", bufs=1, space="PSUM") as ppool:
              for n in range(NCH):
                psums = [
                    ppool.tile([P, FD], F32, tag=f"pj{m}", name=f"pj{m}") for m in range(8)
                ]
                for k in range(NK):
                    htk = ht_pool.tile([P, FD], F32R, tag="ht")
                    nc.sync.dma_start(
                        out=htk[:],
                        in_=hT[bass.ts(k, P), bass.ts(n, FD)],
                    )
                    for m in range(8):
                        nc.tensor.matmul(
                            psums[m][:],
                            (lhsT_for(m, k)),
                            (htk[:]),
                            start=(k == 0),
                            stop=(k == NK - 1),
                        )
                for m in range(8):
                    nc.scalar.copy(out=evict_target(m, n), in_=psums[m][:])

            # ---------------- rotary ----------------
            # raw rope layout rows: [h0_x1(32) h0_x2(32) h1_x1(32) h1_x2(32)]
            # swapped: block b <- block b^1;  out = raw*A + swapped*B
            sw = wpool.tile([P, S], F32, tag="sw")
            t1 = wpool.tile([P, S], F32, tag="t1")
            t2 = wpool.tile([P, S], F32, tag="t2")
            for raw, dst in ((qr_raw, q_rope), (kr_raw, k_rope)):
                for b in range(4):
                    src = b ^ 1
                    nc.sync.dma_start(
                        out=sw[bass.ts(b, 32), :], in_=raw[bass.ts(src, 32), :]
                    )
                nc.vector.tensor_mul(t1[:], raw[:], ra_sb[:])
                nc.vector.tensor_mul(t2[:], sw[:], rb_sb[:])
                nc.vector.tensor_add(dst[:], t1[:], t2[:])

            # ---------------- vT -> v transpose ----------------
            idt = wpool.tile([P, P], F32, tag="idt")
            make_identity(nc, idt[:])
            tpool = pctx.enter_context(
                tc.tile_pool(name="tp_psum", bufs=4, space="PSUM")
            )
            for t in range(NT):
                for h in range(HPC):
                    tp = tpool.tile([P, P], F32, tag="tp")
                    nc.tensor.transpose(
                        tp[:], vt_sb[h][:, bass.ts(t, P)], idt[:]
                    )
                    nc.vector.tensor_copy(
                        out=v_sb[:, t, bass.ts(h, DV)], in_=tp[:]
                    )

        # ---------------- attention + Wo ----------------
        with ExitStack() as actx:
            apool = actx.enter_context(tc.tile_pool(name="apool", bufs=1))
            wo_sb = apool.tile([P, HPC, HID], F32R, tag="wo")
            nc.sync.dma_start(out=wo_sb[:], in_=wo.rearrange("(c p) o -> p c o", p=P))
            ones_col_f = apool.tile([P, 1], F32, tag="ones_cf")
            ones_row_f = apool.tile([1, P], F32, tag="ones_rf")
            nc.vector.memset(ones_col_f[:], 1.0)
            nc.vector.memset(ones_row_f[:], 1.0)
            ones_col = apool.tile([P, 1], F32R, tag="ones_c")
            ones_row = apool.tile([1, P], F32R, tag="ones_r")
            nc.vector.tensor_copy(out=ones_col[:], in_=ones_col_f[:])
            nc.vector.tensor_copy(out=ones_row[:], in_=ones_row_f[:])
            a_sb = [apool.tile([P, S], F32R, tag=f"a{h}", name=f"a{h}") for h in range(HPC)]

            exp_pool = actx.enter_context(tc.tile_pool(name="exp", bufs=16))
            rs_pool = actx.enter_context(tc.tile_pool(name="rs", bufs=2))
            out_pool = actx.enter_context(tc.tile_pool(name="osb", bufs=2))
            sc_psum = actx.enter_context(
                tc.tile_pool(name="sc_psum", bufs=2, space="PSUM")
            )
            av_psum = actx.enter_context(
                tc.tile_pool(name="av_psum", bufs=3, space="PSUM")
            )
            sm_psum = actx.enter_context(
                tc.tile_pool(name="sm_psum", bufs=2, space="PSUM")
            )

            for i in range(NCH):          # query chunk (512 positions)
                isl = bass.ts(i, FD)
                for h in range(HPC):
                    rsl = bass.ds(h * DR, DR)   # rope rows for this head
                    exps = []
                    for j in range(NT):
                        jsl = bass.ts(j, P)
                        sp = sc_psum.tile([P, FD], F32, tag="sc")
                        nc.tensor.matmul(
                            sp[:],
                            (k_nope[h][:, jsl]),
                            (q_nope[h][:, isl]),
                            start=True,
                            stop=False,
                        )
                        nc.tensor.matmul(
                            sp[:],
                            (k_rope[rsl, jsl]),
                            (q_rope[rsl, isl]),
                            start=False,
                            stop=True,
                        )
                        e = exp_pool.tile([P, FD], F32R, tag="exp", name="exp_t")
                        nc.scalar.activation(
                            e[:], sp[:], mybir.ActivationFunctionType.Exp
                        )
                        exps.append(e)
                    avp = av_psum.tile([P, FD], F32, tag="av")
                    smp = sm_psum.tile([1, FD], F32, tag="sm")
                    for j in range(NT):
                        nc.tensor.matmul(
                            avp[:],
                            (v_sb[:, j, bass.ts(h, DV)]),
                            (exps[j][:]),
                            start=(j == 0),
                            stop=(j == NT - 1),
                        )
                        nc.tensor.matmul(
                            smp[:],
                            (ones_col[:]),
                            (exps[j][:]),
                            start=(j == 0),
                            stop=(j == NT - 1),
                        )
                    rsum = rs_pool.tile([1, FD], F32R, tag="rsum")
                    with nc.allow_low_precision(reason="f32r softmax scale"):
                        nc.vector.reciprocal(rsum[:], smp[:])
                    bcp = sc_psum.tile([P, FD], F32, tag="sc")
                    nc.tensor.matmul(
                        bcp[:], (ones_row[:]), (rsum[:]), start=True, stop=True
                    )
                    rbc = rs_pool.tile([P, FD], F32, tag="rbc")
                    nc.vector.tensor_copy(out=rbc[:], in_=bcp[:])
                    # normalized attn output for this head/chunk: [dv, i]
                    nc.vector.tensor_mul(a_sb[h][:, isl], avp[:], rbc[:])

                # Wo for the 4 row-tiles of this chunk
                for t in range(4):
                    it = i * 4 + t
                    osb = out_pool.tile([P, HID], F32, tag="osb")
                    for oc in range(NCH):
                        wop = av_psum.tile([P, FD], F32, tag="av")
                        for h in range(HPC):
                            nc.tensor.matmul(
                                wop[:],
                                (a_sb[h][:, bass.ts(it, P)]),
                                (wo_sb[:, h, bass.ts(oc, FD)]),
                                start=(h == 0),
                                stop=(h == HPC - 1),
                            )
                        nc.vector.tensor_copy(out=osb[:, bass.ts(oc, FD)], in_=wop[:])
                    nc.sync.dma_start(out=out[bass.ts(it, P), :], in_=osb[:])

    nc.compile()
    return nc


_NC = None


def _get_program():
    global _NC
    if _NC is None:
        _NC = _build_program()
    return _NC


def _rope_tables():
    e = np.arange(0, DR, 2, dtype=np.float32) / np.float32(DR)
    inv_freq = (np.float32(1.0) / np.power(np.float32(ROPE_BASE), e)).astype(
        np.float32
    )
    t = np.arange(S, dtype=np.float32)
    freqs = t[:, None] * inv_freq[None, :]          # [S, 32]
    cos = np.cos(freqs).astype(np.float32).T        # [32, S]
    sin = np.sin(freqs).astype(np.float32).T
    # raw rope rows per head-half: [x1(32) x2(32)] ; out = raw*A + swapped*B
    # out_x1 = x1*cos - x2*sin ; out_x2 = x2*cos + x1*sin
    a = np.concatenate([cos, cos, cos, cos], axis=0)        # [128, S]
    b = np.concatenate([-sin, sin, -sin, sin], axis=0)
    return np.ascontiguousarray(a), np.ascontiguousarray(b)


def _in_maps(hidden_states, Wq, Wk, Wv, Wo):
    hs = np.asarray(hidden_states, dtype=np.float32)
    Wq = np.asarray(Wq, dtype=np.float32)
    Wk = np.asarray(Wk, dtype=np.float32)
    Wv = np.asarray(Wv, dtype=np.float32)
    Wo = np.asarray(Wo, dtype=np.float32)
    hT = np.ascontiguousarray(hs[0].T)
    ra, rb = _rope_tables()
    sn = np.float32(1.0 / math.sqrt(DN))
    sr = np.float32(1.0 / math.sqrt(DR))
    maps = []
    for c in range(NCORES):
        h0, h1 = HPC * c, HPC * c + 1
        rows = []
        for h, scale in ((h0, sn), (h1, sn)):
            rows.append(Wq[h * DN:(h + 1) * DN] * scale)
        for h in (h0, h1):
            rows.append(Wq[H * DN + h * DR: H * DN + (h + 1) * DR] * sr)
        wq_c = np.ascontiguousarray(np.concatenate(rows, axis=0).T)
        rows = [Wk[h * DN:(h + 1) * DN] for h in (h0, h1)]
        rows += [Wk[H * DN + h * DR: H * DN + (h + 1) * DR] for h in (h0, h1)]
        wk_c = np.ascontiguousarray(np.concatenate(rows, axis=0).T)
        wv_c = np.ascontiguousarray(Wv[h0 * DV:(h1 + 1) * DV].T)
        wo_c = np.ascontiguousarray(Wo[:, h0 * DV:(h1 + 1) * DV].T)
        maps.append(
            dict(hT=hT, wq=wq_c, wk=wk_c, wv=wv_c, wo=wo_c, ra=ra, rb=rb)
        )
    return maps


def run(hidden_states, Wq, Wk, Wv, Wo, **spmd_kwargs):
    """Build+run on the 8 NeuronCores; returns (output, BassKernelResults)."""
    nc = _get_program()
    maps = _in_maps(hidden_states, Wq, Wk, Wv, Wo)
    res = run_bass_kernel_spmd(nc, maps, list(range(NCORES)), **spmd_kwargs)
    acc = np.zeros((S, HID), dtype=np.float32)
    for r in res.results:
        acc += r["out"]
    B = np.asarray(hidden_states).shape[0]
    return acc.reshape(B, S, HID), res


def kernel(hidden_states, Wq, Wk, Wv, Wo):
    out, _ = run(hidden_states, Wq, Wk, Wv, Wo)
    return out


# revision 11
# speedup vs baseline: 1.4886x; 1.4886x over previous
"""Trainium2 Bass kernel for nn_CustomAttention (MLA-style attention, f32).

Reference computation (B=1, S=2048, hidden=2048, H=16 heads):
    q = h @ Wq.T ; k = h @ Wk.T ; v = h @ Wv.T
    split q/k into nope (16x128) + rope (16x64), apply rotary to rope parts,
    scores = qn@kn.T/sqrt(128) + qr@kr.T/sqrt(64) ; attn = softmax(scores)
    out = (attn @ v) @ Wo.T

Sharding: tensor-parallel over heads.  Each of the 8 NeuronCores computes
2 heads end-to-end (its slice of Wq/Wk/Wv and Wo columns) and produces a
partial [S, hidden] output; host sums the 8 partials.

Device-side layout (per core, everything transposed so no on-chip
transposes of activations are needed):
    hT   [hid, S]   hidden_states transposed, bf16 (moving operand of projections)
    wq   [hid, 384] = [nope_h0(128) nope_h1(128) rope_h0(64) rope_h1(64)]
                      columns of Wq_c.T (bf16), nope cols pre-scaled 1/sqrt(128),
                      rope cols pre-scaled 1/sqrt(64)
    wk   [hid, 384] same layout, unscaled (bf16)
    wv   [hid, 256] Wv_c.T (bf16)
    wo   [256, hid] Wo[:, c-slice].T (bf16)
    ra/rb [128, S]  rotary coefficient tables, f32 (see _rope_tables)

Projections compute qT/kT ([dim, S]) and vT; v is PE-transposed back to
[S, dv].  Scores are computed transposed (sT[j, i] = k_j . q_i) so the
attn @ v contraction runs without transposing the softmax output.  The
rope contribution uses K=128 matmuls over both heads' rope rows with the
*other* head's rows zeroed on the q side (K=64 matmuls are ~2x slower).
Softmax row sums come from an all-ones [128,128] stationary matmul that
also broadcasts the sums to every partition; softmax normalization is
folded into the PSUM eviction of attn@v.  Matmuls run in bf16 with f32
PSUM accumulation; softmax/exp and all reductions stay f32.
"""

import math
from contextlib import ExitStack

import numpy as np
import ml_dtypes

import concourse.bass as bass
import concourse.tile as tile
from concourse import bacc, mybir
from concourse.bass_utils import run_bass_kernel_spmd
from concourse.kernels.tile_matmul import make_identity

H = 16
DN = 128
DR = 64
DV = 128
HID = 2048
S = 2048
ROPE_BASE = 10000.0

NCORES = 8
HPC = H // NCORES          # heads per core = 2
QC = HPC * (DN + DR)       # per-core q/k out dim = 384
VC = HPC * DV              # per-core v out dim = 256
P = 128
NK = HID // P              # 16 contraction chunks
FD = 512                   # matmul free dim / psum bank width (f32)
NCH = S // FD              # 4 column chunks of S
NT = S // P                # 16 row tiles of S

F32 = mybir.dt.float32
BF16 = mybir.dt.bfloat16
Exp = mybir.ActivationFunctionType.Exp


def _build_program():
    nc = bacc.Bacc("TRN2", debug=False, num_devices=NCORES)

    hT = nc.dram_tensor("hT", [HID, S], BF16, kind="ExternalInput").ap()
    wq = nc.dram_tensor("wq", [HID, QC], BF16, kind="ExternalInput").ap()
    wk = nc.dram_tensor("wk", [HID, QC], BF16, kind="ExternalInput").ap()
    wv = nc.dram_tensor("wv", [HID, VC], BF16, kind="ExternalInput").ap()
    wo = nc.dram_tensor("wo", [VC, HID], BF16, kind="ExternalInput").ap()
    ra = nc.dram_tensor("ra", [P, S], F32, kind="ExternalInput").ap()
    rb = nc.dram_tensor("rb", [P, S], F32, kind="ExternalInput").ap()
    out = nc.dram_tensor("out", [S, HID], F32, kind="ExternalOutput").ap()

    with tile.TileContext(nc) as tc, ExitStack() as ctx:
        nc = tc.nc
        persist = ctx.enter_context(tc.tile_pool(name="persist", bufs=1))
        q_nope = [persist.tile([P, S], BF16, tag=f"qn{h}", name=f"qn{h}")
                  for h in range(HPC)]
        k_nope = [persist.tile([P, S], BF16, tag=f"kn{h}", name=f"kn{h}")
                  for h in range(HPC)]
        # q rope per head, [128, S] with the other head's 64 rows zeroed
        q_rope_z = [persist.tile([P, S], BF16, tag=f"qz{h}", name=f"qz{h}")
                    for h in range(HPC)]
        k_rope = persist.tile([P, S], BF16, tag="kr")  # rows 0:64 h0, 64:128 h1
        v_sb = persist.tile([P, NT, VC], BF16, tag="v")

        for h in range(HPC):
            nc.vector.memset(q_rope_z[h][:], 0.0)

        # ---------------- projection phase ----------------
        with ExitStack() as pctx:
            wpool = pctx.enter_context(tc.tile_pool(name="wpool", bufs=1))
            wq_sb = wpool.tile([P, NK, QC], BF16, tag="wq")
            wk_sb = wpool.tile([P, NK, QC], BF16, tag="wk")
            wv_sb = wpool.tile([P, NK, VC], BF16, tag="wv")
            wqr = wq.rearrange("(k p) m -> p k m", p=P)
            wkr = wk.rearrange("(k p) m -> p k m", p=P)
            wvr = wv.rearrange("(k p) m -> p k m", p=P)
            # split per k-tile so the first matmuls don't wait on whole-tensor
            # weight DMAs
            for k in range(NK):
                nc.sync.dma_start(out=wq_sb[:, k, :], in_=wqr[:, k, :])
                nc.sync.dma_start(out=wk_sb[:, k, :], in_=wkr[:, k, :])
                nc.sync.dma_start(out=wv_sb[:, k, :], in_=wvr[:, k, :])

            ra_sb = wpool.tile([P, S], F32, tag="ra")
            rb_sb = wpool.tile([P, S], F32, tag="rb")

            qr_raw = wpool.tile([P, S], F32, tag="qr_raw")
            kr_raw = wpool.tile([P, S], F32, tag="kr_raw")
            vt_sb = [wpool.tile([P, S], F32, tag=f"vt{h}", name=f"vt{h}")
                     for h in range(HPC)]
            idt = wpool.tile([P, P], F32, tag="idt")

            ht_pool = pctx.enter_context(tc.tile_pool(name="ht", bufs=3))
            sw_pool = pctx.enter_context(tc.tile_pool(name="swp", bufs=2))
            rt_pool = pctx.enter_context(tc.tile_pool(name="rtp", bufs=2))

            # m: 0,1 q nope; 2 q rope; 3,4 k nope; 5 k rope; 6,7 vT
            def evict_target(m, n):
                sl = bass.ts(n, FD)
                if m < 2:
                    return q_nope[m][:, sl]
                if m == 2:
                    return qr_raw[:, sl]
                if m < 5:
                    return k_nope[m - 3][:, sl]
                if m == 5:
                    return kr_raw[:, sl]
                return vt_sb[m - 6][:, sl]

            def lhsT_for(m, k):
                if m < 3:
                    return wq_sb[:, k, bass.ts(m, P)]
                if m < 6:
                    return wk_sb[:, k, bass.ts(m - 3, P)]
                return wv_sb[:, k, bass.ts(m - 6, P)]

            with tc.tile_pool(name="proj_psum", bufs=1, space="PSUM") as ppool:
                for n in range(NCH):
                    nsl = bass.ts(n, FD)
                    psums = [ppool.tile([P, FD], F32, tag=f"pj{m}",
                                        name=f"pj{m}") for m in range(8)]
                    for k in range(NK):
                        htk = ht_pool.tile([P, FD], BF16, tag="ht")
                        nc.sync.dma_start(
                            out=htk[:], in_=hT[bass.ts(k, P), nsl]
                        )
                        for m in range(8):
                            nc.tensor.matmul(
                                psums[m][:],
                                lhsT_for(m, k),
                                htk[:],
                                start=(k == 0),
                                stop=(k == NK - 1),
                            )
                    for m in range(8):
                        nc.scalar.copy(out=evict_target(m, n), in_=psums[m][:])

                    if n == 0:
                        # rope tables only needed from here on; don't let these
                        # DMAs delay the first hT/weight loads
                        nc.sync.dma_start(out=ra_sb[:], in_=ra)
                        nc.sync.dma_start(out=rb_sb[:], in_=rb)
                        make_identity(nc, idt[:])

                    # ---- rotary for this chunk ----
                    # raw rows: [h0_x1(32) h0_x2(32) h1_x1(32) h1_x2(32)]
                    # swapped: 32-row block b <- block b^1; out = raw*A + sw*B
                    for raw, is_q in ((qr_raw, True), (kr_raw, False)):
                        swt = sw_pool.tile([P, FD], F32, tag="sw", name="sw")
                        for b in range(4):
                            nc.sync.dma_start(
                                out=swt[bass.ts(b, 32), :],
                                in_=raw[bass.ts(b ^ 1, 32), nsl],
                            )
                        t1 = rt_pool.tile([P, FD], F32, tag="t1", name="t1")
                        t2 = rt_pool.tile([P, FD], F32, tag="t2", name="t2")
                        nc.vector.tensor_mul(t1[:], raw[:, nsl], ra_sb[:, nsl])
                        nc.vector.tensor_mul(t2[:], swt[:], rb_sb[:, nsl])
                        if is_q:
                            for h in range(HPC):
                                hs = bass.ds(h * DR, DR)
                                nc.vector.tensor_add(
                                    q_rope_z[h][hs, nsl], t1[hs, :], t2[hs, :]
                                )
                        else:
                            nc.vector.tensor_add(k_rope[:, nsl], t1[:], t2[:])

                    # ---- vT -> v transpose for this chunk (4 j-tiles) ----
                    for t in range(4):
                        jt = n * 4 + t
                        for h in range(HPC):
                            tp = ppool.tile([P, P], F32, tag=f"pj{6 + h}",
                                            name="tp")
                            nc.tensor.transpose(
                                tp[:], vt_sb[h][:, bass.ts(jt, P)], idt[:]
                            )
                            nc.vector.tensor_copy(
                                out=v_sb[:, jt, bass.ts(h, DV)], in_=tp[:]
                            )

        # ---------------- attention + Wo ----------------
        with ExitStack() as actx:
            apool = actx.enter_context(tc.tile_pool(name="apool", bufs=1))
            wo_sb = apool.tile([P, HPC, HID], BF16, tag="wo")
            wor = wo.rearrange("(c p) o -> p c o", p=P)
            for c in range(HPC):
                nc.sync.dma_start(out=wo_sb[:, c, :], in_=wor[:, c, :])
            ones_mat = apool.tile([P, P], BF16, tag="ones")
            nc.vector.memset(ones_mat[:], 1.0)
            a_sb = [apool.tile([P, S], BF16, tag=f"a{h}", name=f"a{h}")
                    for h in range(HPC)]

            exp_pool = actx.enter_context(tc.tile_pool(name="exp", bufs=18))
            rs_pool = actx.enter_context(tc.tile_pool(name="rs", bufs=2))
            out_pool = actx.enter_context(tc.tile_pool(name="osb", bufs=3))
            sc_psum = actx.enter_context(
                tc.tile_pool(name="sc_psum", bufs=2, space="PSUM")
            )
            av_psum = actx.enter_context(
                tc.tile_pool(name="av_psum", bufs=3, space="PSUM")
            )
            sm_psum = actx.enter_context(
                tc.tile_pool(name="sm_psum", bufs=2, space="PSUM")
            )

            for i in range(NCH):          # query chunk (512 positions)
                isl = bass.ts(i, FD)
                for h in range(HPC):
                    exps = []
                    for j in range(NT):
                        jsl = bass.ts(j, P)
                        sp = sc_psum.tile([P, FD], F32, tag="sc", name="sc")
                        nc.tensor.matmul(
                            sp[:], k_nope[h][:, jsl], q_nope[h][:, isl],
                            start=True, stop=False,
                        )
                        nc.tensor.matmul(
                            sp[:], k_rope[:, jsl], q_rope_z[h][:, isl],
                            start=False, stop=True,
                        )
                        e = exp_pool.tile([P, FD], BF16, tag="exp",
                                          name="exp_t")
                        nc.scalar.activation(e[:], sp[:], Exp)
                        exps.append(e)
                    avp = av_psum.tile([P, FD], F32, tag="av", name="avp")
                    smp = sm_psum.tile([P, FD], F32, tag="sm", name="smp")
                    for j in range(NT):
                        nc.tensor.matmul(
                            avp[:], v_sb[:, j, bass.ts(h, DV)], exps[j][:],
                            start=(j == 0), stop=(j == NT - 1),
                        )
                        nc.tensor.matmul(
                            smp[:], ones_mat[:], exps[j][:],
                            start=(j == 0), stop=(j == NT - 1),
                        )
                    # smp rows are all identical = column sums; reciprocal then
                    # fold normalization into the PSUM eviction of attn@v
                    rbc = rs_pool.tile([P, FD], F32, tag="rbc", name="rbc")
                    nc.vector.reciprocal(rbc[:], smp[:])
                    nc.vector.tensor_mul(a_sb[h][:, isl], avp[:], rbc[:])

                # Wo for the 4 row-tiles of this chunk
                for t in range(4):
                    it = i * 4 + t
                    osb = out_pool.tile([P, HID], F32, tag="osb", name="osb")
                    for oc in range(NCH):
                        wop = av_psum.tile([P, FD], F32, tag="av", name="wop")
                        for h in range(HPC):
                            nc.tensor.matmul(
                                wop[:],
                                a_sb[h][:, bass.ts(it, P)],
                                wo_sb[:, h, bass.ts(oc, FD)],
                                start=(h == 0),
                                stop=(h == HPC - 1),
                            )
                        nc.vector.tensor_copy(
                            out=osb[:, bass.ts(oc, FD)], in_=wop[:]
                        )
                    nc.sync.dma_start(out=out[bass.ts(it, P), :], in_=osb[:])

    nc.compile()
    return nc


_NC = None


def _get_program():
    global _NC
    if _NC is None:
        _NC = _build_program()
    return _NC


def _rope_tables():
    e = np.arange(0, DR, 2, dtype=np.float32) / np.float32(DR)
    inv_freq = (np.float32(1.0) / np.power(np.float32(ROPE_BASE), e)).astype(
        np.float32
    )
    t = np.arange(S, dtype=np.float32)
    freqs = t[:, None] * inv_freq[None, :]          # [S, 32]
    cos = np.cos(freqs).astype(np.float32).T        # [32, S]
    sin = np.sin(freqs).astype(np.float32).T
    # raw rope rows per head-half: [x1(32) x2(32)] ; out = raw*A + swapped*B
    # out_x1 = x1*cos - x2*sin ; out_x2 = x2*cos + x1*sin
    a = np.concatenate([cos, cos, cos, cos], axis=0)        # [128, S]
    b = np.concatenate([-sin, sin, -sin, sin], axis=0)
    return np.ascontiguousarray(a), np.ascontiguousarray(b)


def _in_maps(hidden_states, Wq, Wk, Wv, Wo):
    bf = ml_dtypes.bfloat16
    hs = np.asarray(hidden_states, dtype=np.float32)
    Wq = np.asarray(Wq, dtype=np.float32)
    Wk = np.asarray(Wk, dtype=np.float32)
    Wv = np.asarray(Wv, dtype=np.float32)
    Wo = np.asarray(Wo, dtype=np.float32)
    hT = np.ascontiguousarray(hs[0].T.astype(bf))
    ra, rb = _rope_tables()
    sn = np.float32(1.0 / math.sqrt(DN))
    sr = np.float32(1.0 / math.sqrt(DR))
    maps = []
    for c in range(NCORES):
        h0, h1 = HPC * c, HPC * c + 1
        rows = [Wq[h * DN:(h + 1) * DN] * sn for h in (h0, h1)]
        rows += [Wq[H * DN + h * DR: H * DN + (h + 1) * DR] * sr
                 for h in (h0, h1)]
        wq_c = np.ascontiguousarray(np.concatenate(rows, axis=0).T.astype(bf))
        rows = [Wk[h * DN:(h + 1) * DN] for h in (h0, h1)]
        rows += [Wk[H * DN + h * DR: H * DN + (h + 1) * DR] for h in (h0, h1)]
        wk_c = np.ascontiguousarray(np.concatenate(rows, axis=0).T.astype(bf))
        wv_c = np.ascontiguousarray(Wv[h0 * DV:(h1 + 1) * DV].T.astype(bf))
        wo_c = np.ascontiguousarray(Wo[:, h0 * DV:(h1 + 1) * DV].T.astype(bf))
        maps.append(
            dict(hT=hT, wq=wq_c, wk=wk_c, wv=wv_c, wo=wo_c, ra=ra, rb=rb)
        )
    return maps


def run(hidden_states, Wq, Wk, Wv, Wo, **spmd_kwargs):
    """Build+run on the 8 NeuronCores; returns (output, BassKernelResults)."""
    nc = _get_program()
    maps = _in_maps(hidden_states, Wq, Wk, Wv, Wo)
    res = run_bass_kernel_spmd(nc, maps, list(range(NCORES)), **spmd_kwargs)
    acc = np.zeros((S, HID), dtype=np.float32)
    for r in res.results:
        acc += r["out"]
    B = np.asarray(hidden_states).shape[0]
    return acc.reshape(B, S, HID), res


def kernel(hidden_states, Wq, Wk, Wv, Wo):
    out, _ = run(hidden_states, Wq, Wk, Wv, Wo)
    return out


# revision 12
# speedup vs baseline: 1.5596x; 1.0477x over previous
"""Trainium2 Bass kernel for nn_CustomAttention (MLA-style attention, f32).

Reference computation (B=1, S=2048, hidden=2048, H=16 heads):
    q = h @ Wq.T ; k = h @ Wk.T ; v = h @ Wv.T
    split q/k into nope (16x128) + rope (16x64), apply rotary to rope parts,
    scores = qn@kn.T/sqrt(128) + qr@kr.T/sqrt(64) ; attn = softmax(scores)
    out = (attn @ v) @ Wo.T

Sharding: tensor-parallel over heads.  Each of the 8 NeuronCores computes
2 heads end-to-end (its slice of Wq/Wk/Wv and Wo columns) and produces a
partial [S, hidden] output; host sums the 8 partials.

Device-side layout (per core, everything transposed so no on-chip
transposes of activations are needed):
    hT   [hid, S]   hidden_states transposed, bf16 (moving operand of projections)
    wq   [hid, 384] = [nope_h0(128) nope_h1(128) rope_h0(64) rope_h1(64)]
                      columns of Wq_c.T (bf16), nope cols pre-scaled 1/sqrt(128),
                      rope cols pre-scaled 1/sqrt(64)
    wk   [hid, 384] same layout, unscaled (bf16)
    wv   [hid, 256] Wv_c.T (bf16)
    wo   [256, hid] Wo[:, c-slice].T (bf16)
    ra/rb [128, S]  rotary coefficient tables, f32 (see _rope_tables)

Projections compute qT/kT ([dim, S]) and vT; v is PE-transposed back to
[S, dv].  Scores are computed transposed (sT[j, i] = k_j . q_i) so the
attn @ v contraction runs without transposing the softmax output.  The
rope contribution uses K=128 matmuls over both heads' rope rows with the
*other* head's rows zeroed on the q side (K=64 matmuls are ~2x slower).
Softmax row sums come from an all-ones [128,128] stationary matmul that
also broadcasts the sums to every partition; softmax normalization is
folded into the PSUM eviction of attn@v.  Matmuls run in bf16 with f32
PSUM accumulation; softmax/exp and all reductions stay f32.
"""

import math
from contextlib import ExitStack

import numpy as np
import ml_dtypes

import concourse.bass as bass
import concourse.tile as tile
from concourse import bacc, mybir
from concourse.bass_utils import run_bass_kernel_spmd
from concourse.kernels.tile_matmul import make_identity

H = 16
DN = 128
DR = 64
DV = 128
HID = 2048
S = 2048
ROPE_BASE = 10000.0

NCORES = 8
HPC = H // NCORES          # heads per core = 2
QC = HPC * (DN + DR)       # per-core q/k out dim = 384
VC = HPC * DV              # per-core v out dim = 256
P = 128
NK = HID // P              # 16 contraction chunks
FD = 512                   # matmul free dim / psum bank width (f32)
NCH = S // FD              # 4 column chunks of S
NT = S // P                # 16 row tiles of S

F32 = mybir.dt.float32
BF16 = mybir.dt.bfloat16
Exp = mybir.ActivationFunctionType.Exp


def _build_program():
    nc = bacc.Bacc("TRN2", debug=False, num_devices=NCORES)

    hT = nc.dram_tensor("hT", [HID, S], BF16, kind="ExternalInput").ap()
    wq = nc.dram_tensor("wq", [HID, QC], BF16, kind="ExternalInput").ap()
    wk = nc.dram_tensor("wk", [HID, QC], BF16, kind="ExternalInput").ap()
    wv = nc.dram_tensor("wv", [HID, VC], BF16, kind="ExternalInput").ap()
    wo = nc.dram_tensor("wo", [VC, HID], BF16, kind="ExternalInput").ap()
    ra = nc.dram_tensor("ra", [P, S], F32, kind="ExternalInput").ap()
    rb = nc.dram_tensor("rb", [P, S], F32, kind="ExternalInput").ap()
    out = nc.dram_tensor("out", [S, HID], F32, kind="ExternalOutput").ap()

    with tile.TileContext(nc) as tc, ExitStack() as ctx:
        nc = tc.nc
        persist = ctx.enter_context(tc.tile_pool(name="persist", bufs=1))
        q_nope = [persist.tile([P, S], BF16, tag=f"qn{h}", name=f"qn{h}")
                  for h in range(HPC)]
        k_nope = [persist.tile([P, S], BF16, tag=f"kn{h}", name=f"kn{h}")
                  for h in range(HPC)]
        # q rope per head, [128, S] with the other head's 64 rows zeroed
        q_rope_z = [persist.tile([P, S], BF16, tag=f"qz{h}", name=f"qz{h}")
                    for h in range(HPC)]
        k_rope = persist.tile([P, S], BF16, tag="kr")  # rows 0:64 h0, 64:128 h1
        v_sb = persist.tile([P, NT, VC], BF16, tag="v")

        for h in range(HPC):
            nc.vector.memset(q_rope_z[h][:], 0.0)

        # ---------------- projection phase ----------------
        with ExitStack() as pctx:
            wpool = pctx.enter_context(tc.tile_pool(name="wpool", bufs=1))
            wq_sb = wpool.tile([P, NK, QC], BF16, tag="wq")
            wk_sb = wpool.tile([P, NK, QC], BF16, tag="wk")
            wv_sb = wpool.tile([P, NK, VC], BF16, tag="wv")
            wqr = wq.rearrange("(k p) m -> p k m", p=P)
            wkr = wk.rearrange("(k p) m -> p k m", p=P)
            wvr = wv.rearrange("(k p) m -> p k m", p=P)

            ra_sb = wpool.tile([P, S], F32, tag="ra")
            rb_sb = wpool.tile([P, S], F32, tag="rb")

            qr_raw = wpool.tile([P, S], F32, tag="qr_raw")
            kr_raw = wpool.tile([P, S], F32, tag="kr_raw")
            vt_sb = [wpool.tile([P, S], F32, tag=f"vt{h}", name=f"vt{h}")
                     for h in range(HPC)]
            idt = wpool.tile([P, P], F32, tag="idt")

            ht_pool = pctx.enter_context(tc.tile_pool(name="ht", bufs=3))
            sw_pool = pctx.enter_context(tc.tile_pool(name="swp", bufs=2))
            rt_pool = pctx.enter_context(tc.tile_pool(name="rtp", bufs=2))

            # m: 0,1 q nope; 2 q rope; 3,4 k nope; 5 k rope; 6,7 vT
            def evict_target(m, n):
                sl = bass.ts(n, FD)
                if m < 2:
                    return q_nope[m][:, sl]
                if m == 2:
                    return qr_raw[:, sl]
                if m < 5:
                    return k_nope[m - 3][:, sl]
                if m == 5:
                    return kr_raw[:, sl]
                return vt_sb[m - 6][:, sl]

            def lhsT_for(m, k):
                if m < 3:
                    return wq_sb[:, k, bass.ts(m, P)]
                if m < 6:
                    return wk_sb[:, k, bass.ts(m - 3, P)]
                return wv_sb[:, k, bass.ts(m - 6, P)]

            with tc.tile_pool(name="proj_psum", bufs=1, space="PSUM") as ppool:
                for n in range(NCH):
                    nsl = bass.ts(n, FD)
                    psums = [ppool.tile([P, FD], F32, tag=f"pj{m}",
                                        name=f"pj{m}") for m in range(8)]
                    for k in range(NK):
                        if n == 0:
                            # interleave weight k-tile loads with the hT loads
                            # so they land on different DMA queues in parallel
                            nc.sync.dma_start(out=wq_sb[:, k, :], in_=wqr[:, k, :])
                            nc.sync.dma_start(out=wk_sb[:, k, :], in_=wkr[:, k, :])
                            nc.sync.dma_start(out=wv_sb[:, k, :], in_=wvr[:, k, :])
                        htk = ht_pool.tile([P, FD], BF16, tag="ht")
                        nc.sync.dma_start(
                            out=htk[:], in_=hT[bass.ts(k, P), nsl]
                        )
                        for m in range(8):
                            nc.tensor.matmul(
                                psums[m][:],
                                lhsT_for(m, k),
                                htk[:],
                                start=(k == 0),
                                stop=(k == NK - 1),
                            )
                    for m in range(8):
                        if m % 2 == 0:
                            nc.scalar.copy(
                                out=evict_target(m, n), in_=psums[m][:]
                            )
                        else:
                            nc.vector.tensor_copy(
                                out=evict_target(m, n), in_=psums[m][:]
                            )

                    if n == 0:
                        # rope tables only needed from here on; don't let these
                        # DMAs delay the first hT/weight loads
                        nc.sync.dma_start(out=ra_sb[:], in_=ra)
                        nc.sync.dma_start(out=rb_sb[:], in_=rb)
                        make_identity(nc, idt[:])

                    # ---- rotary for this chunk ----
                    # raw rows: [h0_x1(32) h0_x2(32) h1_x1(32) h1_x2(32)]
                    # swapped: 32-row block b <- block b^1; out = raw*A + sw*B
                    for raw, is_q in ((qr_raw, True), (kr_raw, False)):
                        swt = sw_pool.tile([P, FD], F32, tag="sw", name="sw")
                        for b in range(4):
                            nc.sync.dma_start(
                                out=swt[bass.ts(b, 32), :],
                                in_=raw[bass.ts(b ^ 1, 32), nsl],
                            )
                        t1 = rt_pool.tile([P, FD], F32, tag="t1", name="t1")
                        t2 = rt_pool.tile([P, FD], F32, tag="t2", name="t2")
                        nc.vector.tensor_mul(t1[:], raw[:, nsl], ra_sb[:, nsl])
                        nc.vector.tensor_mul(t2[:], swt[:], rb_sb[:, nsl])
                        if is_q:
                            for h in range(HPC):
                                hs = bass.ds(h * DR, DR)
                                nc.vector.tensor_add(
                                    q_rope_z[h][hs, nsl], t1[hs, :], t2[hs, :]
                                )
                        else:
                            nc.vector.tensor_add(k_rope[:, nsl], t1[:], t2[:])

                    # ---- vT -> v transpose for this chunk (4 j-tiles) ----
                    for t in range(4):
                        jt = n * 4 + t
                        for h in range(HPC):
                            tp = ppool.tile([P, P], F32, tag=f"pj{6 + h}",
                                            name="tp")
                            nc.tensor.transpose(
                                tp[:], vt_sb[h][:, bass.ts(jt, P)], idt[:]
                            )
                            nc.vector.tensor_copy(
                                out=v_sb[:, jt, bass.ts(h, DV)], in_=tp[:]
                            )

        # ---------------- attention + Wo ----------------
        with ExitStack() as actx:
            apool = actx.enter_context(tc.tile_pool(name="apool", bufs=1))
            wo_sb = apool.tile([P, HPC, HID], BF16, tag="wo")
            wor = wo.rearrange("(c p) o -> p c o", p=P)
            for c in range(HPC):
                nc.sync.dma_start(out=wo_sb[:, c, :], in_=wor[:, c, :])
            ones_mat = apool.tile([P, P], BF16, tag="ones")
            nc.vector.memset(ones_mat[:], 1.0)
            a_sb = [apool.tile([P, S], BF16, tag=f"a{h}", name=f"a{h}")
                    for h in range(HPC)]

            exp_pool = actx.enter_context(tc.tile_pool(name="exp", bufs=18))
            rs_pool = actx.enter_context(tc.tile_pool(name="rs", bufs=2))
            out_pool = actx.enter_context(tc.tile_pool(name="osb", bufs=3))
            sc_psum = actx.enter_context(
                tc.tile_pool(name="sc_psum", bufs=2, space="PSUM")
            )
            av_psum = actx.enter_context(
                tc.tile_pool(name="av_psum", bufs=2, space="PSUM")
            )
            wo_psum = actx.enter_context(
                tc.tile_pool(name="wo_psum", bufs=2, space="PSUM")
            )
            sm_psum = actx.enter_context(
                tc.tile_pool(name="sm_psum", bufs=2, space="PSUM")
            )

            for i in range(NCH):          # query chunk (512 positions)
                isl = bass.ts(i, FD)
                for h in range(HPC):
                    exps = []
                    for j in range(NT):
                        jsl = bass.ts(j, P)
                        sp = sc_psum.tile([P, FD], F32, tag="sc", name="sc")
                        nc.tensor.matmul(
                            sp[:], k_nope[h][:, jsl], q_nope[h][:, isl],
                            start=True, stop=False,
                        )
                        nc.tensor.matmul(
                            sp[:], k_rope[:, jsl], q_rope_z[h][:, isl],
                            start=False, stop=True,
                        )
                        e = exp_pool.tile([P, FD], BF16, tag="exp",
                                          name="exp_t")
                        nc.scalar.activation(e[:], sp[:], Exp)
                        exps.append(e)
                    avp = av_psum.tile([P, FD], F32, tag="av", name="avp")
                    smp = sm_psum.tile([P, FD], F32, tag="sm", name="smp")
                    for j in range(NT):
                        nc.tensor.matmul(
                            avp[:], v_sb[:, j, bass.ts(h, DV)], exps[j][:],
                            start=(j == 0), stop=(j == NT - 1),
                        )
                        nc.tensor.matmul(
                            smp[:], ones_mat[:], exps[j][:],
                            start=(j == 0), stop=(j == NT - 1),
                        )
                    # smp rows are all identical = column sums; reciprocal then
                    # fold normalization into the PSUM eviction of attn@v
                    rbc = rs_pool.tile([P, FD], F32, tag="rbc", name="rbc")
                    nc.vector.reciprocal(rbc[:], smp[:])
                    nc.vector.tensor_mul(a_sb[h][:, isl], avp[:], rbc[:])

                # Wo for the 4 row-tiles of this chunk
                for t in range(4):
                    it = i * 4 + t
                    osb = out_pool.tile([P, HID], F32, tag="osb", name="osb")
                    for oc in range(NCH):
                        wop = wo_psum.tile([P, FD], F32, tag="wo", name="wop")
                        for h in range(HPC):
                            nc.tensor.matmul(
                                wop[:],
                                a_sb[h][:, bass.ts(it, P)],
                                wo_sb[:, h, bass.ts(oc, FD)],
                                start=(h == 0),
                                stop=(h == HPC - 1),
                            )
                        nc.vector.tensor_copy(
                            out=osb[:, bass.ts(oc, FD)], in_=wop[:]
                        )
                    nc.sync.dma_start(out=out[bass.ts(it, P), :], in_=osb[:])

    nc.compile()
    return nc


_NC = None


def _get_program():
    global _NC
    if _NC is None:
        _NC = _build_program()
    return _NC


def _rope_tables():
    e = np.arange(0, DR, 2, dtype=np.float32) / np.float32(DR)
    inv_freq = (np.float32(1.0) / np.power(np.float32(ROPE_BASE), e)).astype(
        np.float32
    )
    t = np.arange(S, dtype=np.float32)
    freqs = t[:, None] * inv_freq[None, :]          # [S, 32]
    cos = np.cos(freqs).astype(np.float32).T        # [32, S]
    sin = np.sin(freqs).astype(np.float32).T
    # raw rope rows per head-half: [x1(32) x2(32)] ; out = raw*A + swapped*B
    # out_x1 = x1*cos - x2*sin ; out_x2 = x2*cos + x1*sin
    a = np.concatenate([cos, cos, cos, cos], axis=0)        # [128, S]
    b = np.concatenate([-sin, sin, -sin, sin], axis=0)
    return np.ascontiguousarray(a), np.ascontiguousarray(b)


def _in_maps(hidden_states, Wq, Wk, Wv, Wo):
    bf = ml_dtypes.bfloat16
    hs = np.asarray(hidden_states, dtype=np.float32)
    Wq = np.asarray(Wq, dtype=np.float32)
    Wk = np.asarray(Wk, dtype=np.float32)
    Wv = np.asarray(Wv, dtype=np.float32)
    Wo = np.asarray(Wo, dtype=np.float32)
    hT = np.ascontiguousarray(hs[0].T.astype(bf))
    ra, rb = _rope_tables()
    sn = np.float32(1.0 / math.sqrt(DN))
    sr = np.float32(1.0 / math.sqrt(DR))
    maps = []
    for c in range(NCORES):
        h0, h1 = HPC * c, HPC * c + 1
        rows = [Wq[h * DN:(h + 1) * DN] * sn for h in (h0, h1)]
        rows += [Wq[H * DN + h * DR: H * DN + (h + 1) * DR] * sr
                 for h in (h0, h1)]
        wq_c = np.ascontiguousarray(np.concatenate(rows, axis=0).T.astype(bf))
        rows = [Wk[h * DN:(h + 1) * DN] for h in (h0, h1)]
        rows += [Wk[H * DN + h * DR: H * DN + (h + 1) * DR] for h in (h0, h1)]
        wk_c = np.ascontiguousarray(np.concatenate(rows, axis=0).T.astype(bf))
        wv_c = np.ascontiguousarray(Wv[h0 * DV:(h1 + 1) * DV].T.astype(bf))
        wo_c = np.ascontiguousarray(Wo[:, h0 * DV:(h1 + 1) * DV].T.astype(bf))
        maps.append(
            dict(hT=hT, wq=wq_c, wk=wk_c, wv=wv_c, wo=wo_c, ra=ra, rb=rb)
        )
    return maps


def run(hidden_states, Wq, Wk, Wv, Wo, **spmd_kwargs):
    """Build+run on the 8 NeuronCores; returns (output, BassKernelResults)."""
    nc = _get_program()
    maps = _in_maps(hidden_states, Wq, Wk, Wv, Wo)
    res = run_bass_kernel_spmd(nc, maps, list(range(NCORES)), **spmd_kwargs)
    acc = np.zeros((S, HID), dtype=np.float32)
    for r in res.results:
        acc += r["out"]
    B = np.asarray(hidden_states).shape[0]
    return acc.reshape(B, S, HID), res


def kernel(hidden_states, Wq, Wk, Wv, Wo):
    out, _ = run(hidden_states, Wq, Wk, Wv, Wo)
    return out


# revision 13
# speedup vs baseline: 1.5840x; 1.0157x over previous
"""Trainium2 Bass kernel for nn_CustomAttention (MLA-style attention, f32).

Reference computation (B=1, S=2048, hidden=2048, H=16 heads):
    q = h @ Wq.T ; k = h @ Wk.T ; v = h @ Wv.T
    split q/k into nope (16x128) + rope (16x64), apply rotary to rope parts,
    scores = qn@kn.T/sqrt(128) + qr@kr.T/sqrt(64) ; attn = softmax(scores)
    out = (attn @ v) @ Wo.T

Sharding: tensor-parallel over heads.  Each of the 8 NeuronCores computes
2 heads end-to-end (its slice of Wq/Wk/Wv and Wo columns) and produces a
partial [S, hidden] output; host sums the 8 partials.

Device-side layout (per core, everything transposed so no on-chip
transposes of activations are needed):
    hT   [hid, S]   hidden_states transposed, bf16 (moving operand of projections)
    wq   [hid, 384] = [nope_h0(128) nope_h1(128) rope_h0(64) rope_h1(64)]
                      columns of Wq_c.T (bf16), nope cols pre-scaled 1/sqrt(128),
                      rope cols pre-scaled 1/sqrt(64)
    wk   [hid, 384] same layout, unscaled (bf16)
    wv   [hid, 256] Wv_c.T (bf16)
    wo   [256, hid] Wo[:, c-slice].T (bf16)
    ra/rb [128, S]  rotary coefficient tables, f32 (see _rope_tables)

Projections compute qT/kT ([dim, S]) and vT; v is PE-transposed back to
[S, dv].  Scores are computed transposed (sT[j, i] = k_j . q_i) so the
attn @ v contraction runs without transposing the softmax output.  The
rope contribution uses K=128 matmuls over both heads' rope rows with the
*other* head's rows zeroed on the q side (K=64 matmuls are ~2x slower).
Softmax row sums come from an all-ones [128,128] stationary matmul that
also broadcasts the sums to every partition; softmax normalization is
folded into the PSUM eviction of attn@v.  Matmuls run in bf16 with f32
PSUM accumulation; softmax/exp and all reductions stay f32.
"""

import math
from contextlib import ExitStack

import numpy as np
import ml_dtypes

import concourse.bass as bass
import concourse.tile as tile
from concourse import bacc, mybir
from concourse.bass_utils import run_bass_kernel_spmd
from concourse.kernels.tile_matmul import make_identity

H = 16
DN = 128
DR = 64
DV = 128
HID = 2048
S = 2048
ROPE_BASE = 10000.0

NCORES = 8
HPC = H // NCORES          # heads per core = 2
QC = HPC * (DN + DR)       # per-core q/k out dim = 384
VC = HPC * DV              # per-core v out dim = 256
P = 128
NK = HID // P              # 16 contraction chunks
FD = 512                   # matmul free dim / psum bank width (f32)
NCH = S // FD              # 4 column chunks of S
NT = S // P                # 16 row tiles of S

F32 = mybir.dt.float32
BF16 = mybir.dt.bfloat16
Exp = mybir.ActivationFunctionType.Exp


def _build_program():
    nc = bacc.Bacc("TRN2", debug=False, num_devices=NCORES)

    hT = nc.dram_tensor("hT", [HID, S], BF16, kind="ExternalInput").ap()
    wq = nc.dram_tensor("wq", [HID, QC], BF16, kind="ExternalInput").ap()
    wk = nc.dram_tensor("wk", [HID, QC], BF16, kind="ExternalInput").ap()
    wv = nc.dram_tensor("wv", [HID, VC], BF16, kind="ExternalInput").ap()
    wo = nc.dram_tensor("wo", [VC, HID], BF16, kind="ExternalInput").ap()
    ra = nc.dram_tensor("ra", [P, S], F32, kind="ExternalInput").ap()
    rb = nc.dram_tensor("rb", [P, S], F32, kind="ExternalInput").ap()
    out = nc.dram_tensor("out", [S, HID], F32, kind="ExternalOutput").ap()

    with tile.TileContext(nc) as tc, ExitStack() as ctx:
        nc = tc.nc
        persist = ctx.enter_context(tc.tile_pool(name="persist", bufs=1))
        q_nope = [persist.tile([P, S], BF16, tag=f"qn{h}", name=f"qn{h}")
                  for h in range(HPC)]
        k_nope = [persist.tile([P, S], BF16, tag=f"kn{h}", name=f"kn{h}")
                  for h in range(HPC)]
        # q rope per head, [128, S] with the other head's 64 rows zeroed
        q_rope_z = [persist.tile([P, S], BF16, tag=f"qz{h}", name=f"qz{h}")
                    for h in range(HPC)]
        k_rope = persist.tile([P, S], BF16, tag="kr")  # rows 0:64 h0, 64:128 h1
        v_sb = persist.tile([P, NT, VC], BF16, tag="v")

        for h in range(HPC):
            nc.vector.memset(q_rope_z[h][:], 0.0)

        # ---------------- projection phase ----------------
        with ExitStack() as pctx:
            wpool = pctx.enter_context(tc.tile_pool(name="wpool", bufs=1))
            wq_sb = wpool.tile([P, NK, QC], BF16, tag="wq")
            wk_sb = wpool.tile([P, NK, QC], BF16, tag="wk")
            wv_sb = wpool.tile([P, NK, VC], BF16, tag="wv")
            wqr = wq.rearrange("(k p) m -> p k m", p=P)
            wkr = wk.rearrange("(k p) m -> p k m", p=P)
            wvr = wv.rearrange("(k p) m -> p k m", p=P)

            ra_sb = wpool.tile([P, S], F32, tag="ra")
            rb_sb = wpool.tile([P, S], F32, tag="rb")

            qr_raw = wpool.tile([P, S], F32, tag="qr_raw")
            kr_raw = wpool.tile([P, S], F32, tag="kr_raw")
            vt_sb = [wpool.tile([P, S], F32, tag=f"vt{h}", name=f"vt{h}")
                     for h in range(HPC)]
            idt = wpool.tile([P, P], F32, tag="idt")

            ht_pool = pctx.enter_context(tc.tile_pool(name="ht", bufs=3))
            sw_pool = pctx.enter_context(tc.tile_pool(name="swp", bufs=2))
            rt_pool = pctx.enter_context(tc.tile_pool(name="rtp", bufs=2))

            # m: 0,1 q nope; 2 q rope; 3,4 k nope; 5 k rope; 6,7 vT
            def evict_target(m, n):
                sl = bass.ts(n, FD)
                if m < 2:
                    return q_nope[m][:, sl]
                if m == 2:
                    return qr_raw[:, sl]
                if m < 5:
                    return k_nope[m - 3][:, sl]
                if m == 5:
                    return kr_raw[:, sl]
                return vt_sb[m - 6][:, sl]

            def lhsT_for(m, k):
                if m < 3:
                    return wq_sb[:, k, bass.ts(m, P)]
                if m < 6:
                    return wk_sb[:, k, bass.ts(m - 3, P)]
                return wv_sb[:, k, bass.ts(m - 6, P)]

            with tc.tile_pool(name="proj_psum", bufs=1, space="PSUM") as ppool:
                for n in range(NCH):
                    nsl = bass.ts(n, FD)
                    psums = [ppool.tile([P, FD], F32, tag=f"pj{m}",
                                        name=f"pj{m}") for m in range(8)]
                    for k in range(NK):
                        if n == 0:
                            # interleave weight k-tile loads with the hT loads
                            # so they land on different DMA queues in parallel
                            nc.sync.dma_start(out=wq_sb[:, k, :], in_=wqr[:, k, :])
                            nc.sync.dma_start(out=wk_sb[:, k, :], in_=wkr[:, k, :])
                            nc.sync.dma_start(out=wv_sb[:, k, :], in_=wvr[:, k, :])
                        htk = ht_pool.tile([P, FD], BF16, tag="ht")
                        nc.sync.dma_start(
                            out=htk[:], in_=hT[bass.ts(k, P), nsl]
                        )
                        for m in range(8):
                            nc.tensor.matmul(
                                psums[m][:],
                                lhsT_for(m, k),
                                htk[:],
                                start=(k == 0),
                                stop=(k == NK - 1),
                            )
                            if k == NK - 1:
                                # evict immediately so the bank frees while
                                # the remaining m-tiles still stream
                                if m % 2 == 0:
                                    nc.scalar.copy(
                                        out=evict_target(m, n),
                                        in_=psums[m][:],
                                    )
                                else:
                                    nc.vector.tensor_copy(
                                        out=evict_target(m, n),
                                        in_=psums[m][:],
                                    )

                    if n == 0:
                        # rope tables only needed from here on; don't let these
                        # DMAs delay the first hT/weight loads
                        nc.sync.dma_start(out=ra_sb[:], in_=ra)
                        nc.sync.dma_start(out=rb_sb[:], in_=rb)
                        make_identity(nc, idt[:])

                    # ---- rotary for this chunk ----
                    # raw rows: [h0_x1(32) h0_x2(32) h1_x1(32) h1_x2(32)]
                    # swapped: 32-row block b <- block b^1; out = raw*A + sw*B
                    for raw, is_q in ((qr_raw, True), (kr_raw, False)):
                        swt = sw_pool.tile([P, FD], F32, tag="sw", name="sw")
                        for b in range(4):
                            nc.sync.dma_start(
                                out=swt[bass.ts(b, 32), :],
                                in_=raw[bass.ts(b ^ 1, 32), nsl],
                            )
                        t1 = rt_pool.tile([P, FD], F32, tag="t1", name="t1")
                        t2 = rt_pool.tile([P, FD], F32, tag="t2", name="t2")
                        nc.vector.tensor_mul(t1[:], raw[:, nsl], ra_sb[:, nsl])
                        nc.vector.tensor_mul(t2[:], swt[:], rb_sb[:, nsl])
                        if is_q:
                            for h in range(HPC):
                                hs = bass.ds(h * DR, DR)
                                nc.vector.tensor_add(
                                    q_rope_z[h][hs, nsl], t1[hs, :], t2[hs, :]
                                )
                        else:
                            nc.vector.tensor_add(k_rope[:, nsl], t1[:], t2[:])

                    # ---- vT -> v transpose for this chunk (4 j-tiles) ----
                    for t in range(4):
                        jt = n * 4 + t
                        for h in range(HPC):
                            tp = ppool.tile([P, P], F32, tag=f"pj{6 + h}",
                                            name="tp")
                            nc.tensor.transpose(
                                tp[:], vt_sb[h][:, bass.ts(jt, P)], idt[:]
                            )
                            nc.vector.tensor_copy(
                                out=v_sb[:, jt, bass.ts(h, DV)], in_=tp[:]
                            )

        # ---------------- attention + Wo ----------------
        with ExitStack() as actx:
            apool = actx.enter_context(tc.tile_pool(name="apool", bufs=1))
            wo_sb = apool.tile([P, HPC, HID], BF16, tag="wo")
            wor = wo.rearrange("(c p) o -> p c o", p=P)
            for c in range(HPC):
                nc.sync.dma_start(out=wo_sb[:, c, :], in_=wor[:, c, :])
            ones_mat = apool.tile([P, P], BF16, tag="ones")
            nc.vector.memset(ones_mat[:], 1.0)
            a_sb = [apool.tile([P, S], BF16, tag=f"a{h}", name=f"a{h}")
                    for h in range(HPC)]

            exp_pool = actx.enter_context(tc.tile_pool(name="exp", bufs=18))
            rs_pool = actx.enter_context(tc.tile_pool(name="rs", bufs=2))
            out_pool = actx.enter_context(tc.tile_pool(name="osb", bufs=3))
            sc_psum = actx.enter_context(
                tc.tile_pool(name="sc_psum", bufs=2, space="PSUM")
            )
            av_psum = actx.enter_context(
                tc.tile_pool(name="av_psum", bufs=2, space="PSUM")
            )
            wo_psum = actx.enter_context(
                tc.tile_pool(name="wo_psum", bufs=2, space="PSUM")
            )
            sm_psum = actx.enter_context(
                tc.tile_pool(name="sm_psum", bufs=2, space="PSUM")
            )

            for i in range(NCH):          # query chunk (512 positions)
                isl = bass.ts(i, FD)
                for h in range(HPC):
                    exps = []
                    for j in range(NT):
                        jsl = bass.ts(j, P)
                        sp = sc_psum.tile([P, FD], F32, tag="sc", name="sc")
                        nc.tensor.matmul(
                            sp[:], k_nope[h][:, jsl], q_nope[h][:, isl],
                            start=True, stop=False,
                        )
                        nc.tensor.matmul(
                            sp[:], k_rope[:, jsl], q_rope_z[h][:, isl],
                            start=False, stop=True,
                        )
                        e = exp_pool.tile([P, FD], BF16, tag="exp",
                                          name="exp_t")
                        nc.scalar.activation(e[:], sp[:], Exp)
                        exps.append(e)
                    avp = av_psum.tile([P, FD], F32, tag="av", name="avp")
                    smp = sm_psum.tile([P, FD], F32, tag="sm", name="smp")
                    for j in range(NT):
                        nc.tensor.matmul(
                            avp[:], v_sb[:, j, bass.ts(h, DV)], exps[j][:],
                            start=(j == 0), stop=(j == NT - 1),
                        )
                        nc.tensor.matmul(
                            smp[:], ones_mat[:], exps[j][:],
                            start=(j == 0), stop=(j == NT - 1),
                        )
                    # smp rows are all identical = column sums; reciprocal then
                    # fold normalization into the PSUM eviction of attn@v
                    rbc = rs_pool.tile([P, FD], F32, tag="rbc", name="rbc")
                    nc.vector.reciprocal(rbc[:], smp[:])
                    nc.vector.tensor_mul(a_sb[h][:, isl], avp[:], rbc[:])

                # Wo pipelined one chunk behind (deps long since ready)
                for t in range(4):
                    if i == 0:
                        break
                    it = (i - 1) * 4 + t
                    osb = out_pool.tile([P, HID], F32, tag="osb", name="osb")
                    for oc in range(NCH):
                        wop = wo_psum.tile([P, FD], F32, tag="wo", name="wop")
                        for h in range(HPC):
                            nc.tensor.matmul(
                                wop[:],
                                a_sb[h][:, bass.ts(it, P)],
                                wo_sb[:, h, bass.ts(oc, FD)],
                                start=(h == 0),
                                stop=(h == HPC - 1),
                            )
                        nc.vector.tensor_copy(
                            out=osb[:, bass.ts(oc, FD)], in_=wop[:]
                        )
                    nc.sync.dma_start(out=out[bass.ts(it, P), :], in_=osb[:])

            for t in range(4):
                it = (NCH - 1) * 4 + t
                osb = out_pool.tile([P, HID], F32, tag="osb", name="osb")
                for oc in range(NCH):
                    wop = wo_psum.tile([P, FD], F32, tag="wo", name="wop")
                    for h in range(HPC):
                        nc.tensor.matmul(
                            wop[:],
                            a_sb[h][:, bass.ts(it, P)],
                            wo_sb[:, h, bass.ts(oc, FD)],
                            start=(h == 0),
                            stop=(h == HPC - 1),
                        )
                    nc.vector.tensor_copy(
                        out=osb[:, bass.ts(oc, FD)], in_=wop[:]
                    )
                nc.sync.dma_start(out=out[bass.ts(it, P), :], in_=osb[:])

    nc.compile()
    return nc


_NC = None


def _get_program():
    global _NC
    if _NC is None:
        _NC = _build_program()
    return _NC


def _rope_tables():
    e = np.arange(0, DR, 2, dtype=np.float32) / np.float32(DR)
    inv_freq = (np.float32(1.0) / np.power(np.float32(ROPE_BASE), e)).astype(
        np.float32
    )
    t = np.arange(S, dtype=np.float32)
    freqs = t[:, None] * inv_freq[None, :]          # [S, 32]
    cos = np.cos(freqs).astype(np.float32).T        # [32, S]
    sin = np.sin(freqs).astype(np.float32).T
    # raw rope rows per head-half: [x1(32) x2(32)] ; out = raw*A + swapped*B
    # out_x1 = x1*cos - x2*sin ; out_x2 = x2*cos + x1*sin
    a = np.concatenate([cos, cos, cos, cos], axis=0)        # [128, S]
    b = np.concatenate([-sin, sin, -sin, sin], axis=0)
    return np.ascontiguousarray(a), np.ascontiguousarray(b)


def _in_maps(hidden_states, Wq, Wk, Wv, Wo):
    bf = ml_dtypes.bfloat16
    hs = np.asarray(hidden_states, dtype=np.float32)
    Wq = np.asarray(Wq, dtype=np.float32)
    Wk = np.asarray(Wk, dtype=np.float32)
    Wv = np.asarray(Wv, dtype=np.float32)
    Wo = np.asarray(Wo, dtype=np.float32)
    hT = np.ascontiguousarray(hs[0].T.astype(bf))
    ra, rb = _rope_tables()
    sn = np.float32(1.0 / math.sqrt(DN))
    sr = np.float32(1.0 / math.sqrt(DR))
    maps = []
    for c in range(NCORES):
        h0, h1 = HPC * c, HPC * c + 1
        rows = [Wq[h * DN:(h + 1) * DN] * sn for h in (h0, h1)]
        rows += [Wq[H * DN + h * DR: H * DN + (h + 1) * DR] * sr
                 for h in (h0, h1)]
        wq_c = np.ascontiguousarray(np.concatenate(rows, axis=0).T.astype(bf))
        rows = [Wk[h * DN:(h + 1) * DN] for h in (h0, h1)]
        rows += [Wk[H * DN + h * DR: H * DN + (h + 1) * DR] for h in (h0, h1)]
        wk_c = np.ascontiguousarray(np.concatenate(rows, axis=0).T.astype(bf))
        wv_c = np.ascontiguousarray(Wv[h0 * DV:(h1 + 1) * DV].T.astype(bf))
        wo_c = np.ascontiguousarray(Wo[:, h0 * DV:(h1 + 1) * DV].T.astype(bf))
        maps.append(
            dict(hT=hT, wq=wq_c, wk=wk_c, wv=wv_c, wo=wo_c, ra=ra, rb=rb)
        )
    return maps


def run(hidden_states, Wq, Wk, Wv, Wo, **spmd_kwargs):
    """Build+run on the 8 NeuronCores; returns (output, BassKernelResults)."""
    nc = _get_program()
    maps = _in_maps(hidden_states, Wq, Wk, Wv, Wo)
    res = run_bass_kernel_spmd(nc, maps, list(range(NCORES)), **spmd_kwargs)
    acc = np.zeros((S, HID), dtype=np.float32)
    for r in res.results:
        acc += r["out"]
    B = np.asarray(hidden_states).shape[0]
    return acc.reshape(B, S, HID), res


def kernel(hidden_states, Wq, Wk, Wv, Wo):
    out, _ = run(hidden_states, Wq, Wk, Wv, Wo)
    return out


# revision 14
# speedup vs baseline: 1.6434x; 1.0375x over previous
"""Trainium2 Bass kernel for nn_CustomAttention (MLA-style attention, f32).

Reference computation (B=1, S=2048, hidden=2048, H=16 heads):
    q = h @ Wq.T ; k = h @ Wk.T ; v = h @ Wv.T
    split q/k into nope (16x128) + rope (16x64), apply rotary to rope parts,
    scores = qn@kn.T/sqrt(128) + qr@kr.T/sqrt(64) ; attn = softmax(scores)
    out = (attn @ v) @ Wo.T

Sharding: tensor-parallel over heads.  Each of the 8 NeuronCores computes
2 heads end-to-end (its slice of Wq/Wk/Wv and Wo columns) and produces a
partial [S, hidden] output; host sums the 8 partials.

Device-side layout (per core, everything transposed so no on-chip
transposes of activations are needed):
    hT   [hid, S]   hidden_states transposed, bf16 (moving operand of projections)
    wq   [hid, 384] = [nope_h0(128) nope_h1(128) rope_h0(64) rope_h1(64)]
                      columns of Wq_c.T (bf16), nope cols pre-scaled 1/sqrt(128),
                      rope cols pre-scaled 1/sqrt(64)
    wk   [hid, 384] same layout, unscaled (bf16)
    wv   [hid, 256] Wv_c.T (bf16)
    wo   [256, hid] Wo[:, c-slice].T (bf16)
    ra/rb [128, S]  rotary coefficient tables, f32 (see _rope_tables)

Projections compute qT/kT ([dim, S]) and vT; v is PE-transposed back to
[S, dv].  Scores are computed transposed (sT[j, i] = k_j . q_i) so the
attn @ v contraction runs without transposing the softmax output.  The
rope contribution uses K=128 matmuls over both heads' rope rows with the
*other* head's rows zeroed on the q side (K=64 matmuls are ~2x slower).
Softmax row sums come from an all-ones [128,128] stationary matmul that
also broadcasts the sums to every partition; softmax normalization is
folded into the PSUM eviction of attn@v.  Matmuls run in bf16 with f32
PSUM accumulation; softmax/exp and all reductions stay f32.
"""

import math
from contextlib import ExitStack

import numpy as np
import ml_dtypes

import concourse.bass as bass
import concourse.tile as tile
from concourse import bacc, mybir
from concourse.bass_utils import run_bass_kernel_spmd
from concourse.kernels.tile_matmul import make_identity

H = 16
DN = 128
DR = 64
DV = 128
HID = 2048
S = 2048
ROPE_BASE = 10000.0

NCORES = 8
HPC = H // NCORES          # heads per core = 2
QC = HPC * (DN + DR)       # per-core q/k out dim = 384
VC = HPC * DV              # per-core v out dim = 256
P = 128
NK = HID // P              # 16 contraction chunks
FD = 512                   # matmul free dim / psum bank width (f32)
NCH = S // FD              # 4 column chunks of S
NT = S // P                # 16 row tiles of S

F32 = mybir.dt.float32
BF16 = mybir.dt.bfloat16
Exp = mybir.ActivationFunctionType.Exp


def _build_program():
    nc = bacc.Bacc("TRN2", debug=False, num_devices=NCORES)

    hT = nc.dram_tensor("hT", [HID, S], BF16, kind="ExternalInput").ap()
    wq = nc.dram_tensor("wq", [HID, QC], BF16, kind="ExternalInput").ap()
    wk = nc.dram_tensor("wk", [HID, QC], BF16, kind="ExternalInput").ap()
    wv = nc.dram_tensor("wv", [HID, VC], BF16, kind="ExternalInput").ap()
    wo = nc.dram_tensor("wo", [VC, HID], BF16, kind="ExternalInput").ap()
    ra = nc.dram_tensor("ra", [P, S], F32, kind="ExternalInput").ap()
    rb = nc.dram_tensor("rb", [P, S], F32, kind="ExternalInput").ap()
    out = nc.dram_tensor("out", [S, HID], F32, kind="ExternalOutput").ap()

    with tile.TileContext(nc) as tc, ExitStack() as ctx:
        nc = tc.nc
        persist = ctx.enter_context(tc.tile_pool(name="persist", bufs=1))
        q_nope = [persist.tile([P, S], BF16, tag=f"qn{h}", name=f"qn{h}")
                  for h in range(HPC)]
        k_nope = [persist.tile([P, S], BF16, tag=f"kn{h}", name=f"kn{h}")
                  for h in range(HPC)]
        # q rope per head, [128, S] with the other head's 64 rows zeroed
        q_rope_z = [persist.tile([P, S], BF16, tag=f"qz{h}", name=f"qz{h}")
                    for h in range(HPC)]
        k_rope = persist.tile([P, S], BF16, tag="kr")  # rows 0:64 h0, 64:128 h1
        v_sb = persist.tile([P, NT, VC], BF16, tag="v")

        for h in range(HPC):
            nc.vector.memset(q_rope_z[h][:], 0.0)

        # ---------------- projection phase ----------------
        with ExitStack() as pctx:
            wpool = pctx.enter_context(tc.tile_pool(name="wpool", bufs=1))
            wq_sb = wpool.tile([P, NK, QC], BF16, tag="wq")
            wk_sb = wpool.tile([P, NK, QC], BF16, tag="wk")
            wv_sb = wpool.tile([P, NK, VC], BF16, tag="wv")
            wqr = wq.rearrange("(k p) m -> p k m", p=P)
            wkr = wk.rearrange("(k p) m -> p k m", p=P)
            wvr = wv.rearrange("(k p) m -> p k m", p=P)

            ra_sb = wpool.tile([P, S], F32, tag="ra")
            rb_sb = wpool.tile([P, S], F32, tag="rb")

            qr_raw = wpool.tile([P, S], F32, tag="qr_raw")
            kr_raw = wpool.tile([P, S], F32, tag="kr_raw")
            vt_sb = [wpool.tile([P, S], F32, tag=f"vt{h}", name=f"vt{h}")
                     for h in range(HPC)]
            idt = wpool.tile([P, P], F32, tag="idt")

            ht_pool = pctx.enter_context(tc.tile_pool(name="ht", bufs=8))
            sw_pool = pctx.enter_context(tc.tile_pool(name="swp", bufs=2))
            rt_pool = pctx.enter_context(tc.tile_pool(name="rtp", bufs=2))

            # m: 0,1 q nope; 2 q rope; 3,4 k nope; 5 k rope; 6,7 vT
            def evict_target(m, n):
                sl = bass.ts(n, FD)
                if m < 2:
                    return q_nope[m][:, sl]
                if m == 2:
                    return qr_raw[:, sl]
                if m < 5:
                    return k_nope[m - 3][:, sl]
                if m == 5:
                    return kr_raw[:, sl]
                return vt_sb[m - 6][:, sl]

            def lhsT_for(m, k):
                if m < 3:
                    return wq_sb[:, k, bass.ts(m, P)]
                if m < 6:
                    return wk_sb[:, k, bass.ts(m - 3, P)]
                return wv_sb[:, k, bass.ts(m - 6, P)]

            with tc.tile_pool(name="proj_psum", bufs=1, space="PSUM") as ppool:
                for n in range(NCH):
                    nsl = bass.ts(n, FD)
                    psums = [ppool.tile([P, FD], F32, tag=f"pj{m}",
                                        name=f"pj{m}") for m in range(8)]
                    for k in range(NK):
                        if n == 0:
                            # interleave weight k-tile loads with the hT loads
                            # so they land on different DMA queues in parallel
                            nc.sync.dma_start(out=wq_sb[:, k, :], in_=wqr[:, k, :])
                            nc.sync.dma_start(out=wk_sb[:, k, :], in_=wkr[:, k, :])
                            nc.sync.dma_start(out=wv_sb[:, k, :], in_=wvr[:, k, :])
                        htk = ht_pool.tile([P, FD], BF16, tag="ht")
                        nc.sync.dma_start(
                            out=htk[:], in_=hT[bass.ts(k, P), nsl]
                        )
                        for m in range(8):
                            nc.tensor.matmul(
                                psums[m][:],
                                lhsT_for(m, k),
                                htk[:],
                                start=(k == 0),
                                stop=(k == NK - 1),
                            )
                            if k == NK - 1:
                                # evict immediately so the bank frees while
                                # the remaining m-tiles still stream
                                if m % 2 == 0:
                                    nc.scalar.copy(
                                        out=evict_target(m, n),
                                        in_=psums[m][:],
                                    )
                                else:
                                    nc.vector.tensor_copy(
                                        out=evict_target(m, n),
                                        in_=psums[m][:],
                                    )

                    if n == 0:
                        # rope tables only needed from here on; don't let these
                        # DMAs delay the first hT/weight loads
                        nc.sync.dma_start(out=ra_sb[:], in_=ra)
                        nc.sync.dma_start(out=rb_sb[:], in_=rb)
                        make_identity(nc, idt[:])

                    # ---- rotary for this chunk ----
                    # raw rows: [h0_x1(32) h0_x2(32) h1_x1(32) h1_x2(32)]
                    # swapped: 32-row block b <- block b^1; out = raw*A + sw*B
                    for raw, is_q in ((qr_raw, True), (kr_raw, False)):
                        swt = sw_pool.tile([P, FD], F32, tag="sw", name="sw")
                        for b in range(4):
                            nc.sync.dma_start(
                                out=swt[bass.ts(b, 32), :],
                                in_=raw[bass.ts(b ^ 1, 32), nsl],
                            )
                        t1 = rt_pool.tile([P, FD], F32, tag="t1", name="t1")
                        t2 = rt_pool.tile([P, FD], F32, tag="t2", name="t2")
                        nc.vector.tensor_mul(t1[:], raw[:, nsl], ra_sb[:, nsl])
                        nc.vector.tensor_mul(t2[:], swt[:], rb_sb[:, nsl])
                        if is_q:
                            for h in range(HPC):
                                hs = bass.ds(h * DR, DR)
                                nc.vector.tensor_add(
                                    q_rope_z[h][hs, nsl], t1[hs, :], t2[hs, :]
                                )
                        else:
                            nc.vector.tensor_add(k_rope[:, nsl], t1[:], t2[:])

                    # ---- vT -> v transpose for this chunk (4 j-tiles) ----
                    for t in range(4):
                        jt = n * 4 + t
                        for h in range(HPC):
                            tp = ppool.tile([P, P], F32, tag=f"pj{6 + h}",
                                            name="tp")
                            nc.tensor.transpose(
                                tp[:], vt_sb[h][:, bass.ts(jt, P)], idt[:]
                            )
                            nc.vector.tensor_copy(
                                out=v_sb[:, jt, bass.ts(h, DV)], in_=tp[:]
                            )

        # ---------------- attention + Wo ----------------
        with ExitStack() as actx:
            apool = actx.enter_context(tc.tile_pool(name="apool", bufs=1))
            wo_sb = apool.tile([P, HPC, HID], BF16, tag="wo")
            wor = wo.rearrange("(c p) o -> p c o", p=P)
            for c in range(HPC):
                nc.sync.dma_start(out=wo_sb[:, c, :], in_=wor[:, c, :])
            ones_mat = apool.tile([P, P], BF16, tag="ones")
            nc.vector.memset(ones_mat[:], 1.0)
            a_sb = [apool.tile([P, S], BF16, tag=f"a{h}", name=f"a{h}")
                    for h in range(HPC)]

            exp_pool = actx.enter_context(tc.tile_pool(name="exp", bufs=18))
            rs_pool = actx.enter_context(tc.tile_pool(name="rs", bufs=2))
            out_pool = actx.enter_context(tc.tile_pool(name="osb", bufs=4))
            sc_psum = actx.enter_context(
                tc.tile_pool(name="sc_psum", bufs=2, space="PSUM")
            )
            av_psum = actx.enter_context(
                tc.tile_pool(name="av_psum", bufs=2, space="PSUM")
            )
            wo_psum = actx.enter_context(
                tc.tile_pool(name="wo_psum", bufs=2, space="PSUM")
            )
            sm_psum = actx.enter_context(
                tc.tile_pool(name="sm_psum", bufs=2, space="PSUM")
            )

            for i in range(NCH):          # query chunk (512 positions)
                isl = bass.ts(i, FD)
                for h in range(HPC):
                    exps = []
                    for j in range(NT):
                        jsl = bass.ts(j, P)
                        sp = sc_psum.tile([P, FD], F32, tag="sc", name="sc")
                        nc.tensor.matmul(
                            sp[:], k_nope[h][:, jsl], q_nope[h][:, isl],
                            start=True, stop=False,
                        )
                        nc.tensor.matmul(
                            sp[:], k_rope[:, jsl], q_rope_z[h][:, isl],
                            start=False, stop=True,
                        )
                        e = exp_pool.tile([P, FD], BF16, tag="exp",
                                          name="exp_t")
                        nc.scalar.activation(e[:], sp[:], Exp)
                        exps.append(e)
                    avp = av_psum.tile([P, FD], F32, tag="av", name="avp")
                    smp = sm_psum.tile([P, FD], F32, tag="sm", name="smp")
                    for j in range(NT):
                        nc.tensor.matmul(
                            avp[:], v_sb[:, j, bass.ts(h, DV)], exps[j][:],
                            start=(j == 0), stop=(j == NT - 1),
                        )
                        nc.tensor.matmul(
                            smp[:], ones_mat[:], exps[j][:],
                            start=(j == 0), stop=(j == NT - 1),
                        )
                    # smp rows are all identical = column sums; reciprocal then
                    # fold normalization into the PSUM eviction of attn@v
                    rbc = rs_pool.tile([P, FD], F32, tag="rbc", name="rbc")
                    nc.vector.reciprocal(rbc[:], smp[:])
                    nc.vector.tensor_mul(a_sb[h][:, isl], avp[:], rbc[:])

                # Wo pipelined one chunk behind (deps long since ready)
                for t in range(4):
                    if i == 0:
                        break
                    it = (i - 1) * 4 + t
                    osb = out_pool.tile([P, HID], F32, tag="osb", name="osb")
                    for oc in range(NCH):
                        wop = wo_psum.tile([P, FD], F32, tag="wo", name="wop")
                        for h in range(HPC):
                            nc.tensor.matmul(
                                wop[:],
                                a_sb[h][:, bass.ts(it, P)],
                                wo_sb[:, h, bass.ts(oc, FD)],
                                start=(h == 0),
                                stop=(h == HPC - 1),
                            )
                        nc.vector.tensor_copy(
                            out=osb[:, bass.ts(oc, FD)], in_=wop[:]
                        )
                    for hh in range(2):
                        nc.sync.dma_start(
                            out=out[bass.ts(it, P), bass.ts(hh, HID // 2)],
                            in_=osb[:, bass.ts(hh, HID // 2)],
                        )

            for t in range(4):
                it = (NCH - 1) * 4 + t
                osb = out_pool.tile([P, HID], F32, tag="osb", name="osb")
                for oc in range(NCH):
                    wop = wo_psum.tile([P, FD], F32, tag="wo", name="wop")
                    for h in range(HPC):
                        nc.tensor.matmul(
                            wop[:],
                            a_sb[h][:, bass.ts(it, P)],
                            wo_sb[:, h, bass.ts(oc, FD)],
                            start=(h == 0),
                            stop=(h == HPC - 1),
                        )
                    nc.vector.tensor_copy(
                        out=osb[:, bass.ts(oc, FD)], in_=wop[:]
                    )
                for hh in range(2):
                    nc.sync.dma_start(
                        out=out[bass.ts(it, P), bass.ts(hh, HID // 2)],
                        in_=osb[:, bass.ts(hh, HID // 2)],
                    )

    nc.compile()
    return nc


_NC = None


def _get_program():
    global _NC
    if _NC is None:
        _NC = _build_program()
    return _NC


def _rope_tables():
    e = np.arange(0, DR, 2, dtype=np.float32) / np.float32(DR)
    inv_freq = (np.float32(1.0) / np.power(np.float32(ROPE_BASE), e)).astype(
        np.float32
    )
    t = np.arange(S, dtype=np.float32)
    freqs = t[:, None] * inv_freq[None, :]          # [S, 32]
    cos = np.cos(freqs).astype(np.float32).T        # [32, S]
    sin = np.sin(freqs).astype(np.float32).T
    # raw rope rows per head-half: [x1(32) x2(32)] ; out = raw*A + swapped*B
    # out_x1 = x1*cos - x2*sin ; out_x2 = x2*cos + x1*sin
    a = np.concatenate([cos, cos, cos, cos], axis=0)        # [128, S]
    b = np.concatenate([-sin, sin, -sin, sin], axis=0)
    return np.ascontiguousarray(a), np.ascontiguousarray(b)


def _in_maps(hidden_states, Wq, Wk, Wv, Wo):
    bf = ml_dtypes.bfloat16
    hs = np.asarray(hidden_states, dtype=np.float32)
    Wq = np.asarray(Wq, dtype=np.float32)
    Wk = np.asarray(Wk, dtype=np.float32)
    Wv = np.asarray(Wv, dtype=np.float32)
    Wo = np.asarray(Wo, dtype=np.float32)
    hT = np.ascontiguousarray(hs[0].T.astype(bf))
    ra, rb = _rope_tables()
    sn = np.float32(1.0 / math.sqrt(DN))
    sr = np.float32(1.0 / math.sqrt(DR))
    maps = []
    for c in range(NCORES):
        h0, h1 = HPC * c, HPC * c + 1
        rows = [Wq[h * DN:(h + 1) * DN] * sn for h in (h0, h1)]
        rows += [Wq[H * DN + h * DR: H * DN + (h + 1) * DR] * sr
                 for h in (h0, h1)]
        wq_c = np.ascontiguousarray(np.concatenate(rows, axis=0).T.astype(bf))
        rows = [Wk[h * DN:(h + 1) * DN] for h in (h0, h1)]
        rows += [Wk[H * DN + h * DR: H * DN + (h + 1) * DR] for h in (h0, h1)]
        wk_c = np.ascontiguousarray(np.concatenate(rows, axis=0).T.astype(bf))
        wv_c = np.ascontiguousarray(Wv[h0 * DV:(h1 + 1) * DV].T.astype(bf))
        wo_c = np.ascontiguousarray(Wo[:, h0 * DV:(h1 + 1) * DV].T.astype(bf))
        maps.append(
            dict(hT=hT, wq=wq_c, wk=wk_c, wv=wv_c, wo=wo_c, ra=ra, rb=rb)
        )
    return maps


def run(hidden_states, Wq, Wk, Wv, Wo, **spmd_kwargs):
    """Build+run on the 8 NeuronCores; returns (output, BassKernelResults)."""
    nc = _get_program()
    maps = _in_maps(hidden_states, Wq, Wk, Wv, Wo)
    res = run_bass_kernel_spmd(nc, maps, list(range(NCORES)), **spmd_kwargs)
    acc = np.zeros((S, HID), dtype=np.float32)
    for r in res.results:
        acc += r["out"]
    B = np.asarray(hidden_states).shape[0]
    return acc.reshape(B, S, HID), res


def kernel(hidden_states, Wq, Wk, Wv, Wo):
    out, _ = run(hidden_states, Wq, Wk, Wv, Wo)
    return out
